# revision 27
# baseline (speedup 1.0000x reference)
"""Hard-negative contrastive loss on 8 TRN2 NeuronCores (Bass/Tile).

Reference semantics (B=1024, Q=32, D=512, temp scalar):
    sim[i,j,q] = fusion[i] . target[j,q];  v[i,j] = max_q sim / temp
    loss = mean_i(lse_j(v[i,:]) - v[i,i])
         + 0.5 * mean_i(log(exp(pos) + sum exp(top512 offdiag)) - pos)

Sharding: target rows j are split 128/core. Each core computes its
(1024 x 128) column block of v with fp8e4m3 DoubleRow matmuls
(contraction d on partitions, two 128-chunks per instruction; host
pre-transposes and scales by 4 per side). The jq-block loop is
outermost so after half the blocks every i-tile's first 64 columns are
done: that half is exchanged with a bf16 AllToAll that overlaps the
remaining matmuls; a second AllToAll ships the other half. The q-max
runs on the DVE as one reduce per 4-bank psum group. Core c then holds
full rows c*128..c*128+127 (as 16*dot values; the 1/(16*temp) scale
folds into the exp) and computes per-row losses in the exp domain:
the diagonal is killed in-place (fused with the off-diag row max), so
E = exp(c*(v - m)) has an exactly-zero diagonal; pos is extracted with
an exact one-hot dot; a 6-step bisection on per-row counts finds the
top-512 threshold; acc = sumfull - sum(min(E, mid)) + 512*mid gives
the hard-negative sum with boundary correction. The host averages the
per-row losses (and takes the two logs).
"""
import sys

if "/opt/trn_rl_repo" not in sys.path:
    sys.path.insert(0, "/opt/trn_rl_repo")

import numpy as np

N_CORES = 8
B, Q, D = 1024, 32, 512
JQ = (B // N_CORES) * Q        # 4096 target vectors per core
KC = D // 128                  # 4 contraction chunks
KP = KC // 2                   # 2 DoubleRow chunk-pairs
NBLK = 512                     # jq per psum bank
NB = JQ // NBLK                # 8 jq blocks
JBLK = NBLK // Q               # 16 j columns per psum bank
HCOL = (NB // 2) * JBLK        # 64 j columns per exchange half
N_ITERS = 6                    # bisection update steps in exp domain
NUM_HARD = B // 2              # 512
FP8_SCALE = 4.0                # per-side prescale before fp8 cast

_RUNNER = None


def _build(repeats=1):
    import concourse.bacc as bacc
    import concourse.mybir as mybir
    import concourse.tile as tile

    f32 = mybir.dt.float32
    bf16 = mybir.dt.bfloat16
    fp8 = mybir.dt.float8e4
    Alu = mybir.AluOpType
    Act = mybir.ActivationFunctionType
    X = mybir.AxisListType.X
    DR = mybir.MatmulPerfMode.DoubleRow

    # v values are (FP8_SCALE^2 * dot); fold the correction into the exp
    VS = 1.0 / (FP8_SCALE * FP8_SCALE * 0.07)

    nc = bacc.Bacc(None, target_bir_lowering=False, debug=False,
                   num_devices=N_CORES)

    # host-prepped layouts: partition dim first, contiguous per DMA chunk
    fus_ap = nc.dram_tensor("fusb", [128, KC, B], fp8, kind="ExternalInput").ap()
    tgt_ap = nc.dram_tensor("tgtb", [128, NB, KC, NBLK], fp8,
                            kind="ExternalInput").ap()
    mask8_ap = nc.dram_tensor("mask8", [128, B], bf16, kind="ExternalInput").ap()
    onehn_ap = nc.dram_tensor("onehn", [128, B], bf16, kind="ExternalInput").ap()
    out_ap = nc.dram_tensor("rowloss", [128, 3], f32, kind="ExternalOutput").ap()

    with tile.TileContext(nc) as tc:
        with (
            tc.tile_pool(name="fus", bufs=1) as fus_pool,
            tc.tile_pool(name="tgt", bufs=1) as tgt_pool,
            tc.tile_pool(name="res", bufs=1) as res_pool,
            tc.tile_pool(name="big", bufs=1) as big_pool,
            tc.tile_pool(name="small", bufs=1) as small_pool,
            tc.tile_pool(name="psum", bufs=1, space="PSUM") as psum_pool,
            tc.tile_pool(name="dram", bufs=1, space="DRAM") as dram_pool,
        ):
            # ---------- inputs: contiguous DMAs on parallel queues ----------
            fus = fus_pool.tile([128, KC, B], fp8)
            for k in range(KC):
                nc.sync.dma_start(fus[:, k, :], fus_ap[:, k, :])
            tgt = tgt_pool.tile([128, NB, KC, NBLK], fp8)
            for b in range(NB):
                nc.gpsimd.dma_start(tgt[:, b, :, :], tgt_ap[:, b, :, :])
            mask8 = big_pool.tile([128, B], bf16)
            nc.scalar.dma_start(mask8[:], mask8_ap[:])
            onehn = big_pool.tile([128, B], bf16)
            nc.scalar.dma_start(onehn[:], onehn_ap[:])

            def sm(name, dt=f32):
                return small_pool.tile([128, 1], dt, name=name, tag=name)

            m, mh0, negm, posv, pv0, pv1, epos, sum_nd, sumfull = (
                sm(n) for n in "m mh0 negm posv pv0 pv1 epos sumnd sumfull".split())
            delta, cnt, summin, accp, mid, dummye = (
                sm(n) for n in "delta cnt summin accp mid dummye".split())

            # warm the Act Exp table so the load is off the phase-2 path
            nc.vector.memset(dummye[:], 0.0)
            nc.scalar.activation(dummye[:], dummye[:], Act.Exp)

            # persistent tiles (reused across timing repeats)
            P_sb = res_pool.tile([128, N_CORES, 128], bf16)  # [i_part, i_tile, j]
            p_in = [dram_pool.tile([B, HCOL], bf16, name=f"p_in{h}", tag=f"p_in{h}")
                    for h in range(2)]
            p_out = [dram_pool.tile([B, HCOL], bf16, name=f"p_out{h}", tag=f"p_out{h}")
                     for h in range(2)]
            Vb = big_pool.tile([128, B], bf16)
            Vb4 = Vb.rearrange("p (s h j) -> p s h j", s=N_CORES, h=2)
            E = big_pool.tile([128, B], bf16)
            junk = big_pool.tile([128, B], bf16)
            junkb = big_pool.tile([128, B], bf16)
            oneh4 = oneh.rearrange("p (s h j) -> p s h j", s=N_CORES, h=2)
            onehn4 = onehn.rearrange("p (s h j) -> p s h j", s=N_CORES, h=2)
            junk4 = junk.rearrange("p (s h j) -> p s h j", s=N_CORES, h=2)
            outs = res_pool.tile([128, 3], f32)
            lnpos = outs[:, 2:3]

            def exchange_half(h):
                nc.sync.dma_start(
                    p_in[h].rearrange("(s p) j -> p s j", s=N_CORES),
                    P_sb[:, :, h * HCOL:(h + 1) * HCOL])
                nc.gpsimd.collective_compute(
                    "AllToAll",
                    Alu.bypass,
                    replica_groups=[list(range(N_CORES))],
                    ins=[p_in[h].opt()],
                    outs=[p_out[h].opt()],
                )
                # assemble this half's rows right away (half 0 overlaps
                # the remaining matmuls and the second exchange)
                nc.sync.dma_start(
                    Vb4[:, :, h, :],
                    p_out[h].rearrange("(s p) j -> p s j", s=N_CORES))

            def body():
                nc.vector.memset(mid[:], 0.5)

                # ---------- phase 1: my (1024 x 128) block of v ----------
                # jq-block outer: after blocks 0..3 every i-tile's columns
                # 0..63 are done, so that half exchanges while blocks 4..7
                # compute. Two 4-bank psum tiles (i-tiles 0-3 / 4-7): one
                # reduce per group amortizes the PSUM access overhead 4x,
                # and the groups alternate so the DVE never waits on the PE.
                TG = N_CORES // 2      # i-tiles per psum group
                for b in range(NB):
                    ps = [psum_pool.tile([128, TG * NBLK], f32, name=f"ps{g}",
                                         tag=f"ps{g}") for g in range(2)]
                    for it in range(N_CORES):
                        g, sl = divmod(it, TG)
                        for h in range(KP):
                            nc.tensor.matmul(
                                ps[g][:, sl * NBLK:(sl + 1) * NBLK],
                                fus[:, 2 * h:2 * h + 2, it * 128:(it + 1) * 128],
                                tgt[:, b, 2 * h:2 * h + 2, :],
                                start=(h == 0),
                                stop=(h == KP - 1),
                                perf_mode=DR,
                            )
                    for g in range(2):
                        nc.vector.reduce_max(
                            P_sb[:, g * TG:(g + 1) * TG, b * JBLK:(b + 1) * JBLK],
                            ps[g].rearrange("p (t j q) -> p t j q", t=TG, q=Q),
                            axis=X,
                        )
                    if b == NB // 2 - 1:
                        exchange_half(0)
                exchange_half(1)

                # ---------- phase 2: per-row losses (exp domain) ----------
                # Half h of Vb (and the one-hot masks laid out identically)
                # arrives with exchange h, so the per-half pos-extraction and
                # diag-kill+rowmax on half 0 hide under the second AllToAll.
                XY = mybir.AxisListType.XY
                for h in range(2):
                    # row max of this half (incl. diag: any m >= max works)
                    nc.vector.reduce_max(
                        (mh0 if h == 0 else m)[:], Vb4[:, :, h, :], axis=XY)
                    # pos extracted exactly: one-hot dot along the row (f32
                    # accumulate of a single nonzero product), no cancellation
                    nc.vector.scalar_tensor_tensor(
                        junk4[:, :, h, :], oneh4[:, :, h, :], 1.0,
                        Vb4[:, :, h, :], op0=Alu.mult, op1=Alu.mult,
                        accum_out=(pv0 if h == 0 else pv1)[:])
                    # kill the diagonal so E itself is the off-diag exp tensor
                    nc.vector.scalar_tensor_tensor(
                        Vb4[:, :, h, :], onehn4[:, :, h, :], 1.0,
                        Vb4[:, :, h, :], op0=Alu.mult, op1=Alu.add)

                nc.vector.tensor_tensor(m[:], m[:], mh0[:], op=Alu.max)
                nc.scalar.mul(negm[:], m[:], -VS)
                nc.vector.tensor_add(posv[:], pv0[:], pv1[:])
                # ln(epos) = VS*posv - VS*m; epos = exp of that
                nc.vector.scalar_tensor_tensor(
                    lnpos, posv[:], VS, negm[:], op0=Alu.mult, op1=Alu.add)
                # E = exp(VS*(V - m)) in (0, 1], diag exactly 0;
                # sum_nd = off-diag row sum (Act accumulator)
                nc.scalar.activation(E[:], Vb[:], Act.Exp, bias=negm[:],
                                     scale=VS, accum_out=sum_nd[:])
                nc.scalar.activation(epos[:], lnpos, Act.Exp)
                nc.vector.tensor_add(sumfull[:], sum_nd[:], epos[:])

                # bisection for the top-512 threshold on exps in [0, 1]:
                # mid += 2*step*(cnt>512) - step, step halves each iteration.
                # mid stays a multiple of 2^-7, exactly representable in bf16.
                step = 0.25
                for _ in range(N_ITERS):
                    nc.vector.tensor_scalar(
                        junkb[:], E[:], mid[:], None, op0=Alu.is_gt,
                        op1=Alu.add, accum_out=cnt[:])
                    nc.vector.tensor_scalar(
                        delta[:], cnt[:], float(NUM_HARD), -0.5,
                        op0=Alu.is_gt, op1=Alu.add)
                    nc.vector.scalar_tensor_tensor(
                        mid[:], delta[:], 2.0 * step, mid[:], op0=Alu.mult,
                        op1=Alu.add)
                    step *= 0.5

                # top-512 sum with boundary correction, via the min identity:
                #   sum(min(E, mid)) = sum(E <= mid) + cnt_hi*mid
                #   acc = epos + sum(E > mid) + (512 - cnt_hi)*mid
                #       = sumfull - sum(min(E, mid)) + 512*mid
                # (cnt_hi cancels; mid is bf16-exact so min() rounds nothing)
                nc.vector.tensor_scalar(
                    junkb[:], E[:], mid[:], None, op0=Alu.min, op1=Alu.add,
                    accum_out=summin[:])
                nc.vector.scalar_tensor_tensor(
                    accp[:], mid[:], float(NUM_HARD), sumfull[:],
                    op0=Alu.mult, op1=Alu.add)
                # host finishes: loss_std = ln(sumfull) - lnpos,
                #                loss_hard = ln(acc) - lnpos (m cancels)
                nc.vector.tensor_copy(outs[:, 0:1], sumfull[:])
                nc.vector.tensor_sub(outs[:, 1:2], accp[:], summin[:])

                nc.sync.dma_start(out_ap[:], outs[:])

            for rep in range(repeats):
                if rep:
                    # timing-only (repeats>1): serialize iterations by
                    # making the next rep's matmul input depend on the
                    # previous rep's final output. (0*outs)+fus == fus,
                    # so values are unchanged.
                    nc.vector.scalar_tensor_tensor(
                        fus[:, 0, 0:1], outs[:, 0:1], 0.0, fus[:, 0, 0:1],
                        op0=mybir.AluOpType.mult, op1=mybir.AluOpType.add)
                body()

    nc.compile()
    return nc


def _get_nc():
    global _RUNNER
    if _RUNNER is None:
        _RUNNER = _build()
    return _RUNNER


def make_in_maps(fusion_feats, target_feats, temp):
    import ml_dtypes

    fusion = np.asarray(fusion_feats, dtype=np.float32)
    target = np.asarray(target_feats, dtype=np.float32)
    # the kernel's exp scale assumes temp=0.07; absorb any deviation here
    fscale = np.float32(FP8_SCALE * 0.07 / float(np.asarray(temp)))
    # fusb[p, k, i] = fusion[i, k*128+p] * fscale
    fusb = np.ascontiguousarray(
        (fusion * fscale).T.reshape(KC, 128, B).transpose(1, 0, 2)
    ).astype(ml_dtypes.float8_e4m3)
    rows_per = B // N_CORES
    in_maps = []
    for c in range(N_CORES):
        shard = target[c * rows_per:(c + 1) * rows_per].reshape(JQ, D)
        # tgtb[p, b, k, j] = shard[b*NBLK+j, k*128+p] * FP8_SCALE
        tgtb = np.ascontiguousarray(
            (shard * np.float32(FP8_SCALE)).T
            .reshape(KC, 128, NB, NBLK).transpose(1, 2, 0, 3)
        ).astype(ml_dtypes.float8_e4m3)
        onehot = np.zeros((rows_per, B), dtype=np.float32)
        onehot[np.arange(rows_per), c * rows_per + np.arange(rows_per)] = 1.0
        in_maps.append({"fusb": fusb, "tgtb": tgtb,
                        "oneh": onehot.astype(ml_dtypes.bfloat16),
                        "onehn": (np.float32(-1000.0) * onehot).astype(
                            ml_dtypes.bfloat16)})
    return in_maps


def combine(results):
    rows = np.concatenate([r["rowloss"] for r in results], axis=0)  # (1024, 3)
    loss_std = np.log(rows[:, 0]) - rows[:, 2]
    loss_hard = np.log(rows[:, 1]) - rows[:, 2]
    loss = loss_std.mean(dtype=np.float32) \
        + np.float32(0.5) * loss_hard.mean(dtype=np.float32)
    return np.asarray(loss, dtype=np.float32)


def kernel(fusion_feats, target_feats, temp):
    from concourse import bass_utils

    nc = _get_nc()
    in_maps = make_in_maps(fusion_feats, target_feats, temp)
    res = bass_utils.run_bass_kernel_spmd(nc, in_maps, list(range(N_CORES)))
    return combine(res.results)
